# revision 32
# baseline (speedup 1.0000x reference)
"""Trainium2 Bass kernel for the CfC cell (nn_CfCCell), data-parallel on 8 cores.

Math (per row):
    ff1 = gelu(x_cat @ W_ff1.T + b_ff1)          x_cat = [x, hx]
    ff2 = gelu(ff1 @ W_ff2.T + b_ff2)
    t   = sigmoid(ff2 @ (W_ta+W_tb).T + b_ta+b_tb)      (TS == 1.0)
    ic  = gelu(x @ W_in.T + b_in + input_b)
    rc  = gelu(hx @ W_r.T + r_b)
    out = hx + t * (ic + rc - hx)

Device mapping: batch sharded 8 ways.  All activations are kept
feature-major ([feat, batch] in SBUF) end-to-end: the host pre-transposes
x/hx into bf16 [feat, batch] DRAM arrays, and the output is stored
feature-major and de-transposed on the host.  This removes every PE
transpose and PSUM->SBUF copy from the device kernel.

Engine budget per core (16384 rows):
  ACT  (the bottleneck): 5 layer-activations x 2048-elem instrs = ~160us.
       All biases are zero in this model, so both 128-feature halves of a
       layer share one [128, 2048] fp32 PSUM tile and one ACTIVATE instr.
  PE:  10 contraction chunks x 2 halves @ bf16 = ~140us (no transposes).
  DVE: 4-op combine via fused scalar_tensor_tensor = ~75us.
  DMA: 20 MiB bf16 I/O = ~56us.
sigmoid is computed as 0.5*tanh(z/2)+0.5 so gelu+tanh live in a single
activation-table set (no table reloads).
"""

from contextlib import ExitStack

import ml_dtypes
import numpy as np

import concourse.bacc as bacc
import concourse.bass as bass
import concourse.mybir as mybir
import concourse.tile as tile
from concourse.bass_utils import run_bass_kernel_spmd

AF = mybir.ActivationFunctionType
ALU = mybir.AluOpType
BF16 = mybir.dt.bfloat16
F32 = mybir.dt.float32
FP8 = mybir.dt.float8e4
DR = mybir.MatmulPerfMode.DoubleRow
NP_BF16 = ml_dtypes.bfloat16
NP_FP8 = ml_dtypes.float8_e4m3

B, I, H = 131072, 128, 256
N_CORES = 8
B_CORE = B // N_CORES  # 16384
R = 2048               # slab cols (DMA granularity); R is the test.py knob
UNIT = 1024            # batch cols per PSUM/ACT unit

# layer order; K = contraction chunks of 128
LAYERS = ("ff1", "ic", "rc", "ff2", "tab")
KCH = {"ff1": 3, "ic": 1, "rc": 2, "ff2": 2, "tab": 2}
W_BASE = {}
_acc = 0
for _l in LAYERS:
    W_BASE[_l] = _acc
    _acc += KCH[_l] * 2
N_WCH = _acc  # 20 weight chunks of [128, 128]
BIAS_COL = {(_l, _m): 2 * _i + _m for _i, _l in enumerate(LAYERS) for _m in range(2)}

# fp8 DoubleRow weight pairs: (layer, k-chunk pair) per feature half m.
# hx streams + the ff1->ff2->tab chain run in e4m3; x and ic stay bf16.
DR_PAIRS = (("ff1", (1, 2)), ("rc", (0, 1)), ("ff2", (0, 1)), ("tab", (0, 1)))
P8 = {(_l, _m): 2 * _i + _m for _i, (_l, _) in enumerate(DR_PAIRS)
      for _m in range(2)}
N_W8 = len(DR_PAIRS) * 2  # 8 pairs of [128, 2, 128]


def build_nc(b_core: int = B_CORE, slab: int = R, zero_bias: bool = True) -> bass.Bass:
    n_slab = b_core // slab
    n_unit = slab // UNIT
    assert b_core % slab == 0 and slab % UNIT == 0

    nc = bacc.Bacc("TRN2")
    xT_d = nc.dram_tensor("xT", [128, b_core], BF16, kind="ExternalInput")
    hxT_d = nc.dram_tensor("hxT", [2, 128, b_core], BF16, kind="ExternalInput")
    hxT8_d = nc.dram_tensor("hxT8", [2, 128, b_core], FP8, kind="ExternalInput")
    # weight stacks arrive pre-swizzled partition-major so their DMAs are
    # plain contiguous copies (cheap HWDGE descriptor generation)
    w_d = nc.dram_tensor("wstack", [128, N_WCH * 128], BF16,
                         kind="ExternalInput")
    w8_d = nc.dram_tensor("w8stack", [128, N_W8 * 256], FP8,
                          kind="ExternalInput")
    b_d = nc.dram_tensor("bstack", [128, 10], F32, kind="ExternalInput")
    out_d = nc.dram_tensor("out", [2, 128, b_core], BF16, kind="ExternalOutput")

    with tile.TileContext(nc) as tc, ExitStack() as ctx:
        const = ctx.enter_context(tc.tile_pool(name="const", bufs=1))
        w_sb = const.tile([128, N_WCH * 128], BF16)
        w8_sb = const.tile([128, N_W8 * 256], FP8)
        b_sb = const.tile([128, 10], F32)

        io_x = ctx.enter_context(tc.tile_pool(name="io_x", bufs=3))
        io_hx = ctx.enter_context(tc.tile_pool(name="io_hx", bufs=3))
        io_out = ctx.enter_context(tc.tile_pool(name="io_out", bufs=2))
        acts = ctx.enter_context(tc.tile_pool(name="acts", bufs=3))
        tmp = ctx.enter_context(tc.tile_pool(name="tmp", bufs=2))
        ps = ctx.enter_context(tc.tile_pool(name="ps", bufs=2, space="PSUM"))

        # HAM warm-up: fine-grained dummy PE work (no DMA dependency via
        # memset) covering the whole load ramp, so the first real matmuls
        # run at 2.4 GHz and slot in with <0.3us queue delay.
        dummy = const.tile([128, 512], BF16)
        nc.gpsimd.memset(dummy[:], 0.25)
        warm = ps.tile([128, 2048], F32, tag="ps")
        for i in range(34):
            nc.tensor.matmul(
                warm[:, (i % 16) * 128:(i % 16 + 1) * 128],
                dummy[:, 0:128], dummy[:, 0:128], start=True, stop=True)

        def wchunk(layer, k, m):
            ci = W_BASE[layer] + 2 * k + m
            return w_sb[:, ci * 128:(ci + 1) * 128]

        def w8pair(layer, m):
            pi = P8[(layer, m)]
            return w8_sb[:, pi * 256:(pi + 1) * 256].rearrange(
                "p (t f) -> p t f", t=2)

        def act_drain(layer, func, scale, ps_t, out_sb):
            flat = out_sb[:].rearrange("p a b -> p (a b)")
            if zero_bias:
                nc.scalar.activation(flat, ps_t[:], func, bias=0.0,
                                     scale=scale)
            else:
                for m in range(2):
                    col = BIAS_COL[(layer, m)]
                    nc.scalar.activation(
                        flat[:, m * UNIT:(m + 1) * UNIT],
                        ps_t[:, m * 1024:(m + 1) * 1024], func,
                        bias=b_sb[:, col:col + 1], scale=scale)

        def mm_unit(layer, srcs, func, scale, out_sb):
            """bf16 matmuls for both feature halves into one fp32 PSUM
            tile, then one (or two, if biased) ACTIVATE drains it."""
            K = KCH[layer]
            ps_t = ps.tile([128, 2048], F32, tag="ps")
            for m in range(2):
                for j in range(2):
                    dst = ps_t[:, m * 1024 + j * 512:
                               m * 1024 + (j + 1) * 512]
                    jsl = slice(j * 512, (j + 1) * 512)
                    for k in range(K):
                        nc.tensor.matmul(
                            dst, wchunk(layer, k, m), srcs[k](jsl),
                            start=(k == 0), stop=(k == K - 1))
            act_drain(layer, func, scale, ps_t, out_sb)

        def mm_unit_dr(layer, pair_src, func, scale, out_sb, extra=None):
            """fp8 DoubleRow: one matmul contracts a pair of 128-chunks
            (2 fp8 weights/cell, rhs [128, 2, 512] streams 2 elem/cycle).
            `extra` optionally adds a leading bf16 chunk (ff1's x part)."""
            ps_t = ps.tile([128, 2048], F32, tag="ps")
            for m in range(2):
                for j in range(2):
                    dst = ps_t[:, m * 1024 + j * 512:
                               m * 1024 + (j + 1) * 512]
                    jsl = slice(j * 512, (j + 1) * 512)
                    if extra is not None:
                        k, src = extra
                        nc.tensor.matmul(
                            dst, wchunk(layer, k, m), src(jsl),
                            start=True, stop=False)
                    nc.tensor.matmul(
                        dst, w8pair(layer, m), pair_src(jsl),
                        start=(extra is None), stop=True, perf_mode=DR)
            act_drain(layer, func, scale, ps_t, out_sb)

        def stage_tail(st):
            """tab matmuls + activation, then the rest of the combine and
            the output stores.  Deferred one unit so tab's matmuls never
            stall the ACT queue behind ff2's ACTIVATE."""
            ff2, dT, hxu, hT_u, stores = st
            uu = acts.tile([128, 2, UNIT], BF16, tag="uu")
            mm_unit_dr("tab", lambda sl: ff2[:, :, sl], AF.Tanh, 0.5, uu)
            # out = hx + t*(ic+rc-hx), t = 0.5*u + 0.5
            ti = tmp.tile([128, 2, UNIT], BF16, tag="ti")
            nc.vector.tensor_scalar(ti[:], uu[:], 0.5, 0.5, ALU.mult, ALU.add)
            pT = tmp.tile([128, 2, UNIT], BF16, tag="pT")
            nc.vector.tensor_mul(pT[:], ti[:], dT[:])
            nc.vector.tensor_add(hT_u, pT[:], hxu)
            for dst, src in stores:
                nc.gpsimd.dma_start(dst, src)

        pending = None
        for s in range(n_slab):
            csl = slice(s * slab, (s + 1) * slab)
            xT = io_x.tile([128, slab], BF16, tag="xT")
            # unit-major layout: a unit's [128, 2, UNIT] slice is contiguous,
            # so the DVE combine ops stay in their 2x/4x perf modes
            hxT = io_hx.tile([128, n_unit, 2, UNIT], BF16, tag="hxT")
            hxT8 = io_hx.tile([128, 2, slab], FP8, tag="hxT8")
            if s == 0:
                # trigger order is latency-critical: weights, then unit 0's
                # matmul inputs, then everything else; bf16 hx (combine-only)
                # loads last
                nc.sync.dma_start(w_sb[:], w_d[:])
                nc.sync.dma_start(w8_sb[:], w8_d[:])
                for q in range(n_unit):
                    qsl = slice(q * UNIT, (q + 1) * UNIT)
                    nc.sync.dma_start(xT[:, qsl], xT_d[:, qsl])
                    nc.sync.dma_start(hxT8[:, 0, qsl], hxT8_d[0, :, qsl])
                    nc.sync.dma_start(hxT8[:, 1, qsl], hxT8_d[1, :, qsl])
                for q in range(n_unit):
                    gsl = slice(q * UNIT, (q + 1) * UNIT)
                    nc.sync.dma_start(hxT[:, q, 0, :], hxT_d[0, :, gsl])
                    nc.sync.dma_start(hxT[:, q, 1, :], hxT_d[1, :, gsl])
                nc.sync.dma_start(b_sb[:], b_d[:])
            else:
                nc.sync.dma_start(xT[:], xT_d[:, csl])
                nc.sync.dma_start(hxT8[:, 0, :], hxT8_d[0, :, csl])
                nc.sync.dma_start(hxT8[:, 1, :], hxT8_d[1, :, csl])
                nc.sync.dma_start(
                    hxT[:, :, 0, :],
                    hxT_d[0, :, csl].rearrange("p (u c) -> p u c", u=n_unit))
                nc.sync.dma_start(
                    hxT[:, :, 1, :],
                    hxT_d[1, :, csl].rearrange("p (u c) -> p u c", u=n_unit))
            hT = io_out.tile([128, n_unit, 2, UNIT], BF16, tag="hT")

            for u in range(n_unit):
                u0 = u * UNIT
                x_src = lambda sl, u0=u0: xT[:, u0 + sl.start:u0 + sl.stop]
                hx8_src = lambda sl, u0=u0: hxT8[:, :, u0 + sl.start:u0 + sl.stop]

                ic = acts.tile([128, 2, UNIT], BF16, tag="ic")
                mm_unit("ic", [x_src], AF.Gelu, 1.0, ic)
                ff1 = acts.tile([128, 2, UNIT], FP8, tag="ff1")
                mm_unit_dr("ff1", hx8_src, AF.Gelu, 1.0, ff1,
                           extra=(0, x_src))
                rc = acts.tile([128, 2, UNIT], BF16, tag="rc")
                mm_unit_dr("rc", hx8_src, AF.Gelu, 1.0, rc)
                ff2 = acts.tile([128, 2, UNIT], FP8, tag="ff2")
                mm_unit_dr("ff2", lambda sl: ff1[:, :, sl], AF.Gelu, 1.0, ff2)

                # s/d only need ic/rc/hx — issue them ahead of the deferred
                # tail so the end-of-kernel combine chain is short
                hxu = hxT[:, u, :, :]
                sT = tmp.tile([128, 2, UNIT], BF16, tag="sT")
                nc.vector.tensor_add(sT[:], ic[:], rc[:])
                dT = tmp.tile([128, 2, UNIT], BF16, tag="dT")
                nc.vector.tensor_sub(dT[:], sT[:], hxu)

                if pending is not None:
                    stage_tail(pending)
                gsl = slice(s * slab + u0, s * slab + u0 + UNIT)
                stores = [(out_d[0, :, gsl], hT[:, u, 0, :]),
                          (out_d[1, :, gsl], hT[:, u, 1, :])]
                pending = (ff2, dT, hxu, hT[:, u, :, :], stores)
        stage_tail(pending)
    nc.finalize()
    return nc


_NC_CACHE: dict = {}


def _get_nc(b_core: int, slab: int, zero_bias: bool) -> bass.Bass:
    key = (b_core, slab, zero_bias)
    if key not in _NC_CACHE:
        _NC_CACHE[key] = build_nc(b_core, slab, zero_bias)
    return _NC_CACHE[key]


def _prep_host(W_ff1, b_ff1, W_ff2, b_ff2, W_ta, b_ta, W_tb, b_tb,
               W_in, b_in, input_b, W_r, r_b):
    f32 = lambda a: np.asarray(a, dtype=np.float32)
    weights = {
        "ff1": f32(W_ff1),
        "ic": f32(W_in),
        "rc": f32(W_r),
        "ff2": f32(W_ff2),
        "tab": f32(W_ta) + f32(W_tb),
    }
    biases = {
        "ff1": f32(b_ff1),
        "ic": f32(b_in) + f32(input_b),
        "rc": f32(r_b),
        "ff2": f32(b_ff2),
        "tab": 0.5 * (f32(b_ta) + f32(b_tb)),
    }
    wstack = np.zeros([N_WCH, 128, 128], dtype=NP_BF16)
    for layer in LAYERS:
        W = weights[layer]
        for k in range(KCH[layer]):
            for m in range(2):
                ci = W_BASE[layer] + 2 * k + m
                wstack[ci] = np.ascontiguousarray(
                    W[m * 128:(m + 1) * 128, k * 128:(k + 1) * 128].T
                ).astype(NP_BF16)
    w8stack = np.zeros([N_W8, 128, 256], dtype=NP_FP8)
    for layer, kpair in DR_PAIRS:
        W = weights[layer]
        for m in range(2):
            pi = P8[(layer, m)]
            for t, k in enumerate(kpair):
                w8stack[pi][:, t * 128:(t + 1) * 128] = np.ascontiguousarray(
                    W[m * 128:(m + 1) * 128, k * 128:(k + 1) * 128].T
                ).astype(NP_FP8)
    bstack = np.zeros([128, 10], dtype=np.float32)
    for li, layer in enumerate(LAYERS):
        for m in range(2):
            bstack[:, 2 * li + m] = biases[layer][m * 128:(m + 1) * 128]
    zero_bias = not np.any(bstack)
    # pre-swizzle partition-major for contiguous DMA
    wstack = np.ascontiguousarray(wstack.transpose(1, 0, 2).reshape(128, -1))
    w8stack = np.ascontiguousarray(w8stack.transpose(1, 0, 2).reshape(128, -1))
    return wstack, w8stack, bstack, zero_bias


def _run(inputs: dict, b_core: int = B_CORE, r: int = R, n_cores: int = N_CORES,
         **run_kwargs):
    wstack, w8stack, bstack, zero_bias = _prep_host(
        inputs["W_ff1"], inputs["b_ff1"], inputs["W_ff2"], inputs["b_ff2"],
        inputs["W_ta"], inputs["b_ta"], inputs["W_tb"], inputs["b_tb"],
        inputs["W_in"], inputs["b_in"], inputs["input_b"], inputs["W_r"],
        inputs["r_b"])
    # host-side transpose to feature-major bf16 (+ fp8 copy of hx)
    hx_f32 = np.asarray(inputs["hx"], dtype=np.float32)
    xT = np.asarray(inputs["x"], dtype=NP_BF16).T          # [128, B]
    hxT = hx_f32.astype(NP_BF16).T                         # [256, B]
    n_rows = xT.shape[1]
    hxT = hxT.reshape(2, 128, n_rows)
    hxT8 = hx_f32.astype(NP_FP8).T.reshape(2, 128, n_rows)

    nc = _get_nc(b_core, r, zero_bias)
    in_maps = []
    for c in range(n_cores):
        sl = slice(c * b_core, (c + 1) * b_core)
        in_maps.append({
            "xT": np.ascontiguousarray(xT[:, sl]),
            "hxT": np.ascontiguousarray(hxT[:, :, sl]),
            "hxT8": np.ascontiguousarray(hxT8[:, :, sl]),
            "wstack": wstack,
            "w8stack": w8stack,
            "bstack": bstack,
        })
    res = run_bass_kernel_spmd(nc, in_maps, list(range(n_cores)), **run_kwargs)
    # de-transpose: out_d[m, p, b] is feature (m*128+p) of row b
    outs = [m["out"].transpose(2, 0, 1).reshape(b_core, H).astype(np.float32)
            for m in res.results]
    out = np.concatenate(outs, axis=0)
    return out, res


def kernel(**inputs):
    out, _ = _run(inputs)
    return (out, out)


# revision 33
# speedup vs baseline: 1.0164x; 1.0164x over previous
"""Trainium2 Bass kernel for the CfC cell (nn_CfCCell), data-parallel on 8 cores.

Math (per row):
    ff1 = gelu(x_cat @ W_ff1.T + b_ff1)          x_cat = [x, hx]
    ff2 = gelu(ff1 @ W_ff2.T + b_ff2)
    t   = sigmoid(ff2 @ (W_ta+W_tb).T + b_ta+b_tb)      (TS == 1.0)
    ic  = gelu(x @ W_in.T + b_in + input_b)
    rc  = gelu(hx @ W_r.T + r_b)
    out = hx + t * (ic + rc - hx)

Device mapping: batch sharded 8 ways.  All activations are kept
feature-major ([feat, batch] in SBUF) end-to-end: the host pre-transposes
x/hx into bf16 [feat, batch] DRAM arrays, and the output is stored
feature-major and de-transposed on the host.  This removes every PE
transpose and PSUM->SBUF copy from the device kernel.

Engine budget per core (16384 rows):
  ACT  (the bottleneck): 5 layer-activations x 2048-elem instrs = ~160us.
       All biases are zero in this model, so both 128-feature halves of a
       layer share one [128, 2048] fp32 PSUM tile and one ACTIVATE instr.
  PE:  10 contraction chunks x 2 halves @ bf16 = ~140us (no transposes).
  DVE: 4-op combine via fused scalar_tensor_tensor = ~75us.
  DMA: 20 MiB bf16 I/O = ~56us.
sigmoid is computed as 0.5*tanh(z/2)+0.5 so gelu+tanh live in a single
activation-table set (no table reloads).
"""

from contextlib import ExitStack

import ml_dtypes
import numpy as np

import concourse.bacc as bacc
import concourse.bass as bass
import concourse.mybir as mybir
import concourse.tile as tile
from concourse.bass_utils import run_bass_kernel_spmd

AF = mybir.ActivationFunctionType
ALU = mybir.AluOpType
BF16 = mybir.dt.bfloat16
F32 = mybir.dt.float32
FP8 = mybir.dt.float8e4
DR = mybir.MatmulPerfMode.DoubleRow
NP_BF16 = ml_dtypes.bfloat16
NP_FP8 = ml_dtypes.float8_e4m3

B, I, H = 131072, 128, 256
N_CORES = 8
B_CORE = B // N_CORES  # 16384
R = 2048               # slab cols (DMA granularity); R is the test.py knob
UNIT = 1024            # batch cols per PSUM/ACT unit

# layer order; K = contraction chunks of 128
LAYERS = ("ff1", "ic", "rc", "ff2", "tab")
KCH = {"ff1": 3, "ic": 1, "rc": 2, "ff2": 2, "tab": 2}
W_BASE = {}
_acc = 0
for _l in LAYERS:
    W_BASE[_l] = _acc
    _acc += KCH[_l] * 2
N_WCH = _acc  # 20 weight chunks of [128, 128]
BIAS_COL = {(_l, _m): 2 * _i + _m for _i, _l in enumerate(LAYERS) for _m in range(2)}

# fp8 DoubleRow weight pairs: (layer, k-chunk pair) per feature half m.
# hx streams + the ff1->ff2->tab chain run in e4m3; x and ic stay bf16.
DR_PAIRS = (("ff1", (1, 2)), ("rc", (0, 1)), ("ff2", (0, 1)), ("tab", (0, 1)))
P8 = {(_l, _m): 2 * _i + _m for _i, (_l, _) in enumerate(DR_PAIRS)
      for _m in range(2)}
N_W8 = len(DR_PAIRS) * 2  # 8 pairs of [128, 2, 128]


def build_nc(b_core: int = B_CORE, slab: int = R, zero_bias: bool = True) -> bass.Bass:
    n_slab = b_core // slab
    n_unit = slab // UNIT
    assert b_core % slab == 0 and slab % UNIT == 0

    nc = bacc.Bacc("TRN2")
    xT_d = nc.dram_tensor("xT", [128, b_core], BF16, kind="ExternalInput")
    hxT_d = nc.dram_tensor("hxT", [2, 128, b_core], BF16, kind="ExternalInput")
    hxT8_d = nc.dram_tensor("hxT8", [2, 128, b_core], FP8, kind="ExternalInput")
    # weight stacks arrive pre-swizzled partition-major so their DMAs are
    # plain contiguous copies (cheap HWDGE descriptor generation)
    w_d = nc.dram_tensor("wstack", [128, N_WCH * 128], BF16,
                         kind="ExternalInput")
    w8_d = nc.dram_tensor("w8stack", [128, N_W8 * 256], FP8,
                          kind="ExternalInput")
    b_d = nc.dram_tensor("bstack", [128, 10], F32, kind="ExternalInput")
    out_d = nc.dram_tensor("out", [2, 128, b_core], BF16, kind="ExternalOutput")

    with tile.TileContext(nc) as tc, ExitStack() as ctx:
        const = ctx.enter_context(tc.tile_pool(name="const", bufs=1))
        w_sb = const.tile([128, N_WCH * 128], BF16)
        w8_sb = const.tile([128, N_W8 * 256], FP8)
        b_sb = const.tile([128, 10], F32)

        io_x = ctx.enter_context(tc.tile_pool(name="io_x", bufs=3))
        io_hx = ctx.enter_context(tc.tile_pool(name="io_hx", bufs=3))
        io_out = ctx.enter_context(tc.tile_pool(name="io_out", bufs=2))
        acts = ctx.enter_context(tc.tile_pool(name="acts", bufs=3))
        tmp = ctx.enter_context(tc.tile_pool(name="tmp", bufs=2))
        ps = ctx.enter_context(tc.tile_pool(name="ps", bufs=2, space="PSUM"))

        # HAM warm-up: fine-grained dummy PE work (no DMA dependency via
        # memset) covering the whole load ramp, so the first real matmuls
        # run at 2.4 GHz and slot in with <0.3us queue delay.
        dummy = const.tile([128, 512], BF16)
        nc.gpsimd.memset(dummy[:], 0.25)
        warm = ps.tile([128, 2048], F32, tag="ps")
        for i in range(34):
            nc.tensor.matmul(
                warm[:, (i % 16) * 128:(i % 16 + 1) * 128],
                dummy[:, 0:128], dummy[:, 0:128], start=True, stop=True)

        def wchunk(layer, k, m):
            ci = W_BASE[layer] + 2 * k + m
            return w_sb[:, ci * 128:(ci + 1) * 128]

        def w8pair(layer, m):
            pi = P8[(layer, m)]
            return w8_sb[:, pi * 256:(pi + 1) * 256].rearrange(
                "p (t f) -> p t f", t=2)

        def act_drain(layer, func, scale, ps_t, out_sb):
            flat = out_sb[:].rearrange("p a b -> p (a b)")
            if zero_bias:
                nc.scalar.activation(flat, ps_t[:], func, bias=0.0,
                                     scale=scale)
            else:
                for m in range(2):
                    col = BIAS_COL[(layer, m)]
                    nc.scalar.activation(
                        flat[:, m * UNIT:(m + 1) * UNIT],
                        ps_t[:, m * 1024:(m + 1) * 1024], func,
                        bias=b_sb[:, col:col + 1], scale=scale)

        def mm_unit(layer, srcs, func, scale, out_sb):
            """bf16 matmuls for both feature halves into one fp32 PSUM
            tile, then one (or two, if biased) ACTIVATE drains it."""
            K = KCH[layer]
            ps_t = ps.tile([128, 2048], F32, tag="ps")
            for m in range(2):
                for j in range(2):
                    dst = ps_t[:, m * 1024 + j * 512:
                               m * 1024 + (j + 1) * 512]
                    jsl = slice(j * 512, (j + 1) * 512)
                    for k in range(K):
                        nc.tensor.matmul(
                            dst, wchunk(layer, k, m), srcs[k](jsl),
                            start=(k == 0), stop=(k == K - 1))
            act_drain(layer, func, scale, ps_t, out_sb)

        def mm_unit_dr(layer, pair_src, func, scale, out_sb, extra=None):
            """fp8 DoubleRow: one matmul contracts a pair of 128-chunks
            (2 fp8 weights/cell, rhs [128, 2, 512] streams 2 elem/cycle).
            `extra` optionally adds a leading bf16 chunk (ff1's x part)."""
            ps_t = ps.tile([128, 2048], F32, tag="ps")
            for m in range(2):
                for j in range(2):
                    dst = ps_t[:, m * 1024 + j * 512:
                               m * 1024 + (j + 1) * 512]
                    jsl = slice(j * 512, (j + 1) * 512)
                    if extra is not None:
                        k, src = extra
                        nc.tensor.matmul(
                            dst, wchunk(layer, k, m), src(jsl),
                            start=True, stop=False)
                    nc.tensor.matmul(
                        dst, w8pair(layer, m), pair_src(jsl),
                        start=(extra is None), stop=True, perf_mode=DR)
            act_drain(layer, func, scale, ps_t, out_sb)

        def stage_tail(st):
            """tab matmuls + activation, then the rest of the combine and
            the output stores.  Deferred one unit so tab's matmuls never
            stall the ACT queue behind ff2's ACTIVATE."""
            ff2, dT, hxu, hT_u, stores = st
            uu = acts.tile([128, 2, UNIT], BF16, tag="uu")
            mm_unit_dr("tab", lambda sl: ff2[:, :, sl], AF.Tanh, 0.5, uu)
            # out = hx + t*(ic+rc-hx), t = 0.5*u + 0.5
            ti = tmp.tile([128, 2, UNIT], BF16, tag="ti")
            nc.vector.tensor_scalar(ti[:], uu[:], 0.5, 0.5, ALU.mult, ALU.add)
            pT = tmp.tile([128, 2, UNIT], BF16, tag="pT")
            nc.vector.tensor_mul(pT[:], ti[:], dT[:])
            nc.vector.tensor_add(hT_u, pT[:], hxu)
            for dst, src in stores:
                nc.sync.dma_start(dst, src)

        pending = None
        for s in range(n_slab):
            csl = slice(s * slab, (s + 1) * slab)
            xT = io_x.tile([128, slab], BF16, tag="xT")
            # unit-major layout: a unit's [128, 2, UNIT] slice is contiguous,
            # so the DVE combine ops stay in their 2x/4x perf modes
            hxT = io_hx.tile([128, n_unit, 2, UNIT], BF16, tag="hxT")
            hxT8 = io_hx.tile([128, 2, slab], FP8, tag="hxT8")
            if s == 0:
                # trigger order is latency-critical: weights, then unit 0's
                # matmul inputs, then everything else; bf16 hx (combine-only)
                # loads last
                nc.sync.dma_start(w_sb[:], w_d[:])
                nc.sync.dma_start(w8_sb[:], w8_d[:])
                for q in range(n_unit):
                    qsl = slice(q * UNIT, (q + 1) * UNIT)
                    nc.sync.dma_start(xT[:, qsl], xT_d[:, qsl])
                    nc.sync.dma_start(hxT8[:, 0, qsl], hxT8_d[0, :, qsl])
                    nc.sync.dma_start(hxT8[:, 1, qsl], hxT8_d[1, :, qsl])
                for q in range(n_unit):
                    gsl = slice(q * UNIT, (q + 1) * UNIT)
                    nc.sync.dma_start(hxT[:, q, 0, :], hxT_d[0, :, gsl])
                    nc.sync.dma_start(hxT[:, q, 1, :], hxT_d[1, :, gsl])
                nc.sync.dma_start(b_sb[:], b_d[:])
            else:
                nc.sync.dma_start(xT[:], xT_d[:, csl])
                nc.sync.dma_start(hxT8[:, 0, :], hxT8_d[0, :, csl])
                nc.sync.dma_start(hxT8[:, 1, :], hxT8_d[1, :, csl])
                nc.sync.dma_start(
                    hxT[:, :, 0, :],
                    hxT_d[0, :, csl].rearrange("p (u c) -> p u c", u=n_unit))
                nc.sync.dma_start(
                    hxT[:, :, 1, :],
                    hxT_d[1, :, csl].rearrange("p (u c) -> p u c", u=n_unit))
            hT = io_out.tile([128, n_unit, 2, UNIT], BF16, tag="hT")

            for u in range(n_unit):
                u0 = u * UNIT
                x_src = lambda sl, u0=u0: xT[:, u0 + sl.start:u0 + sl.stop]
                hx8_src = lambda sl, u0=u0: hxT8[:, :, u0 + sl.start:u0 + sl.stop]

                ic = acts.tile([128, 2, UNIT], BF16, tag="ic")
                mm_unit("ic", [x_src], AF.Gelu, 1.0, ic)
                ff1 = acts.tile([128, 2, UNIT], FP8, tag="ff1")
                mm_unit_dr("ff1", hx8_src, AF.Gelu, 1.0, ff1,
                           extra=(0, x_src))
                rc = acts.tile([128, 2, UNIT], BF16, tag="rc")
                mm_unit_dr("rc", hx8_src, AF.Gelu, 1.0, rc)
                ff2 = acts.tile([128, 2, UNIT], FP8, tag="ff2")
                mm_unit_dr("ff2", lambda sl: ff1[:, :, sl], AF.Gelu, 1.0, ff2)

                # s/d only need ic/rc/hx — issue them ahead of the deferred
                # tail so the end-of-kernel combine chain is short
                hxu = hxT[:, u, :, :]
                sT = tmp.tile([128, 2, UNIT], BF16, tag="sT")
                nc.vector.tensor_add(sT[:], ic[:], rc[:])
                dT = tmp.tile([128, 2, UNIT], BF16, tag="dT")
                nc.vector.tensor_sub(dT[:], sT[:], hxu)

                if pending is not None:
                    stage_tail(pending)
                gsl = slice(s * slab + u0, s * slab + u0 + UNIT)
                stores = [(out_d[0, :, gsl], hT[:, u, 0, :]),
                          (out_d[1, :, gsl], hT[:, u, 1, :])]
                pending = (ff2, dT, hxu, hT[:, u, :, :], stores)
        stage_tail(pending)
    nc.finalize()
    return nc


_NC_CACHE: dict = {}


def _get_nc(b_core: int, slab: int, zero_bias: bool) -> bass.Bass:
    key = (b_core, slab, zero_bias)
    if key not in _NC_CACHE:
        _NC_CACHE[key] = build_nc(b_core, slab, zero_bias)
    return _NC_CACHE[key]


def _prep_host(W_ff1, b_ff1, W_ff2, b_ff2, W_ta, b_ta, W_tb, b_tb,
               W_in, b_in, input_b, W_r, r_b):
    f32 = lambda a: np.asarray(a, dtype=np.float32)
    weights = {
        "ff1": f32(W_ff1),
        "ic": f32(W_in),
        "rc": f32(W_r),
        "ff2": f32(W_ff2),
        "tab": f32(W_ta) + f32(W_tb),
    }
    biases = {
        "ff1": f32(b_ff1),
        "ic": f32(b_in) + f32(input_b),
        "rc": f32(r_b),
        "ff2": f32(b_ff2),
        "tab": 0.5 * (f32(b_ta) + f32(b_tb)),
    }
    wstack = np.zeros([N_WCH, 128, 128], dtype=NP_BF16)
    for layer in LAYERS:
        W = weights[layer]
        for k in range(KCH[layer]):
            for m in range(2):
                ci = W_BASE[layer] + 2 * k + m
                wstack[ci] = np.ascontiguousarray(
                    W[m * 128:(m + 1) * 128, k * 128:(k + 1) * 128].T
                ).astype(NP_BF16)
    w8stack = np.zeros([N_W8, 128, 256], dtype=NP_FP8)
    for layer, kpair in DR_PAIRS:
        W = weights[layer]
        for m in range(2):
            pi = P8[(layer, m)]
            for t, k in enumerate(kpair):
                w8stack[pi][:, t * 128:(t + 1) * 128] = np.ascontiguousarray(
                    W[m * 128:(m + 1) * 128, k * 128:(k + 1) * 128].T
                ).astype(NP_FP8)
    bstack = np.zeros([128, 10], dtype=np.float32)
    for li, layer in enumerate(LAYERS):
        for m in range(2):
            bstack[:, 2 * li + m] = biases[layer][m * 128:(m + 1) * 128]
    zero_bias = not np.any(bstack)
    # pre-swizzle partition-major for contiguous DMA
    wstack = np.ascontiguousarray(wstack.transpose(1, 0, 2).reshape(128, -1))
    w8stack = np.ascontiguousarray(w8stack.transpose(1, 0, 2).reshape(128, -1))
    return wstack, w8stack, bstack, zero_bias


def _run(inputs: dict, b_core: int = B_CORE, r: int = R, n_cores: int = N_CORES,
         **run_kwargs):
    wstack, w8stack, bstack, zero_bias = _prep_host(
        inputs["W_ff1"], inputs["b_ff1"], inputs["W_ff2"], inputs["b_ff2"],
        inputs["W_ta"], inputs["b_ta"], inputs["W_tb"], inputs["b_tb"],
        inputs["W_in"], inputs["b_in"], inputs["input_b"], inputs["W_r"],
        inputs["r_b"])
    # host-side transpose to feature-major bf16 (+ fp8 copy of hx)
    hx_f32 = np.asarray(inputs["hx"], dtype=np.float32)
    xT = np.asarray(inputs["x"], dtype=NP_BF16).T          # [128, B]
    hxT = hx_f32.astype(NP_BF16).T                         # [256, B]
    n_rows = xT.shape[1]
    hxT = hxT.reshape(2, 128, n_rows)
    hxT8 = hx_f32.astype(NP_FP8).T.reshape(2, 128, n_rows)

    nc = _get_nc(b_core, r, zero_bias)
    in_maps = []
    for c in range(n_cores):
        sl = slice(c * b_core, (c + 1) * b_core)
        in_maps.append({
            "xT": np.ascontiguousarray(xT[:, sl]),
            "hxT": np.ascontiguousarray(hxT[:, :, sl]),
            "hxT8": np.ascontiguousarray(hxT8[:, :, sl]),
            "wstack": wstack,
            "w8stack": w8stack,
            "bstack": bstack,
        })
    res = run_bass_kernel_spmd(nc, in_maps, list(range(n_cores)), **run_kwargs)
    # de-transpose: out_d[m, p, b] is feature (m*128+p) of row b
    outs = [m["out"].transpose(2, 0, 1).reshape(b_core, H).astype(np.float32)
            for m in res.results]
    out = np.concatenate(outs, axis=0)
    return out, res


def kernel(**inputs):
    out, _ = _run(inputs)
    return (out, out)


# revision 37
# speedup vs baseline: 1.0177x; 1.0013x over previous
"""Trainium2 Bass kernel for the CfC cell (nn_CfCCell), data-parallel on 8 cores.

Math (per row):
    ff1 = gelu(x_cat @ W_ff1.T + b_ff1)          x_cat = [x, hx]
    ff2 = gelu(ff1 @ W_ff2.T + b_ff2)
    t   = sigmoid(ff2 @ (W_ta+W_tb).T + b_ta+b_tb)      (TS == 1.0)
    ic  = gelu(x @ W_in.T + b_in + input_b)
    rc  = gelu(hx @ W_r.T + r_b)
    out = hx + t * (ic + rc - hx)

Device mapping: batch sharded 8 ways.  All activations are kept
feature-major ([feat, batch] in SBUF) end-to-end: the host pre-transposes
x/hx into bf16 [feat, batch] DRAM arrays, and the output is stored
feature-major and de-transposed on the host.  This removes every PE
transpose and PSUM->SBUF copy from the device kernel.

Engine budget per core (16384 rows):
  ACT  (the bottleneck): 5 layer-activations x 2048-elem instrs = ~160us.
       All biases are zero in this model, so both 128-feature halves of a
       layer share one [128, 2048] fp32 PSUM tile and one ACTIVATE instr.
  PE:  10 contraction chunks x 2 halves @ bf16 = ~140us (no transposes).
  DVE: 4-op combine via fused scalar_tensor_tensor = ~75us.
  DMA: 20 MiB bf16 I/O = ~56us.
sigmoid is computed as 0.5*tanh(z/2)+0.5 so gelu+tanh live in a single
activation-table set (no table reloads).
"""

from contextlib import ExitStack

import ml_dtypes
import numpy as np

import concourse.bacc as bacc
import concourse.bass as bass
import concourse.mybir as mybir
import concourse.tile as tile
from concourse.bass_utils import run_bass_kernel_spmd

AF = mybir.ActivationFunctionType
ALU = mybir.AluOpType
BF16 = mybir.dt.bfloat16
F32 = mybir.dt.float32
FP8 = mybir.dt.float8e4
DR = mybir.MatmulPerfMode.DoubleRow
NP_BF16 = ml_dtypes.bfloat16
NP_FP8 = ml_dtypes.float8_e4m3

B, I, H = 131072, 128, 256
N_CORES = 8
B_CORE = B // N_CORES  # 16384
R = 2048               # slab cols (DMA granularity); R is the test.py knob
UNIT = 1024            # batch cols per PSUM/ACT unit

# layer order; K = contraction chunks of 128
LAYERS = ("ff1", "ic", "rc", "ff2", "tab")
KCH = {"ff1": 3, "ic": 1, "rc": 2, "ff2": 2, "tab": 2}
W_BASE = {}
_acc = 0
for _l in LAYERS:
    W_BASE[_l] = _acc
    _acc += KCH[_l] * 2
N_WCH = _acc  # 20 weight chunks of [128, 128]
BIAS_COL = {(_l, _m): 2 * _i + _m for _i, _l in enumerate(LAYERS) for _m in range(2)}

# fp8 DoubleRow weight pairs: (layer, k-chunk pair) per feature half m.
# hx streams + the ff1->ff2->tab chain run in e4m3; x and ic stay bf16.
DR_PAIRS = (("ff1", (1, 2)), ("rc", (0, 1)), ("ff2", (0, 1)), ("tab", (0, 1)))
P8 = {(_l, _m): 2 * _i + _m for _i, (_l, _) in enumerate(DR_PAIRS)
      for _m in range(2)}
N_W8 = len(DR_PAIRS) * 2  # 8 pairs of [128, 2, 128]


def build_nc(b_core: int = B_CORE, slab: int = R, zero_bias: bool = True) -> bass.Bass:
    n_slab = b_core // slab
    n_unit = slab // UNIT
    assert b_core % slab == 0 and slab % UNIT == 0

    nc = bacc.Bacc("TRN2")
    xT_d = nc.dram_tensor("xT", [128, b_core], BF16, kind="ExternalInput")
    hxT_d = nc.dram_tensor("hxT", [2, 128, b_core], BF16, kind="ExternalInput")
    hxT8_d = nc.dram_tensor("hxT8", [2, 128, b_core], FP8, kind="ExternalInput")
    # weight stacks arrive pre-swizzled partition-major so their DMAs are
    # plain contiguous copies (cheap HWDGE descriptor generation)
    w_d = nc.dram_tensor("wstack", [128, N_WCH * 128], BF16,
                         kind="ExternalInput")
    w8_d = nc.dram_tensor("w8stack", [128, N_W8 * 256], FP8,
                          kind="ExternalInput")
    b_d = nc.dram_tensor("bstack", [128, 10], F32, kind="ExternalInput")
    out_d = nc.dram_tensor("out", [2, 128, b_core], BF16, kind="ExternalOutput")

    with tile.TileContext(nc) as tc, ExitStack() as ctx:
        const = ctx.enter_context(tc.tile_pool(name="const", bufs=1))
        w_sb = const.tile([128, N_WCH * 128], BF16)
        w8_sb = const.tile([128, N_W8 * 256], FP8)
        b_sb = const.tile([128, 10], F32)

        io_x = ctx.enter_context(tc.tile_pool(name="io_x", bufs=3))
        io_hx = ctx.enter_context(tc.tile_pool(name="io_hx", bufs=3))
        io_out = ctx.enter_context(tc.tile_pool(name="io_out", bufs=2))
        acts = ctx.enter_context(tc.tile_pool(name="acts", bufs=3))
        tmp = ctx.enter_context(tc.tile_pool(name="tmp", bufs=2))
        ps = ctx.enter_context(tc.tile_pool(name="ps", bufs=2, space="PSUM"))

        # HAM warm-up: fine-grained dummy PE work (no DMA dependency via
        # memset) covering the whole load ramp, so the first real matmuls
        # run at 2.4 GHz and slot in with <0.3us queue delay.
        dummy = const.tile([128, 512], BF16)
        nc.gpsimd.memset(dummy[:], 0.25)
        warm = ps.tile([128, 2048], F32, tag="ps")
        for i in range(34):
            nc.tensor.matmul(
                warm[:, (i % 16) * 128:(i % 16 + 1) * 128],
                dummy[:, 0:128], dummy[:, 0:128], start=True, stop=True)

        def wchunk(layer, k, m):
            ci = W_BASE[layer] + 2 * k + m
            return w_sb[:, ci * 128:(ci + 1) * 128]

        def w8pair(layer, m):
            pi = P8[(layer, m)]
            return w8_sb[:, pi * 256:(pi + 1) * 256].rearrange(
                "p (t f) -> p t f", t=2)

        def act_drain(layer, func, scale, ps_t, out_sb, cols):
            out_ap = out_sb[:, :, 0:cols]
            if zero_bias:
                nc.scalar.activation(
                    out_ap,
                    ps_t[:, 0:2 * cols].rearrange("p (a b) -> p a b", a=2),
                    func, bias=0.0, scale=scale)
            else:
                for m in range(2):
                    col = BIAS_COL[(layer, m)]
                    nc.scalar.activation(
                        out_sb[:, m, 0:cols],
                        ps_t[:, m * cols:(m + 1) * cols], func,
                        bias=b_sb[:, col:col + 1], scale=scale)

        def mm_unit(layer, srcs, func, scale, out_sb, cols):
            """bf16 matmuls for both feature halves into one fp32 PSUM
            tile, then one (or two, if biased) ACTIVATE drains it."""
            K = KCH[layer]
            ps_t = ps.tile([128, 2048], F32, tag="ps")
            for m in range(2):
                for j in range(cols // 512):
                    dst = ps_t[:, m * cols + j * 512:
                               m * cols + (j + 1) * 512]
                    jsl = slice(j * 512, (j + 1) * 512)
                    for k in range(K):
                        nc.tensor.matmul(
                            dst, wchunk(layer, k, m), srcs[k](jsl),
                            start=(k == 0), stop=(k == K - 1))
            act_drain(layer, func, scale, ps_t, out_sb, cols)

        def mm_unit_dr(layer, pair_src, func, scale, out_sb, cols,
                       extra=None):
            """fp8 DoubleRow: one matmul contracts a pair of 128-chunks
            (2 fp8 weights/cell, rhs [128, 2, 512] streams 2 elem/cycle).
            `extra` optionally adds a leading bf16 chunk (ff1's x part)."""
            ps_t = ps.tile([128, 2048], F32, tag="ps")
            for m in range(2):
                for j in range(cols // 512):
                    dst = ps_t[:, m * cols + j * 512:
                               m * cols + (j + 1) * 512]
                    jsl = slice(j * 512, (j + 1) * 512)
                    if extra is not None:
                        k, src = extra
                        nc.tensor.matmul(
                            dst, wchunk(layer, k, m), src(jsl),
                            start=True, stop=False)
                    nc.tensor.matmul(
                        dst, w8pair(layer, m), pair_src(jsl),
                        start=(extra is None), stop=True, perf_mode=DR)
            act_drain(layer, func, scale, ps_t, out_sb, cols)

        def stage_tail(st):
            """tab matmuls + activation, then the rest of the combine and
            the output stores.  Deferred one unit so tab's matmuls never
            stall the ACT queue behind ff2's ACTIVATE."""
            ff2, dT, hxu, hT_u, stores, cols = st
            uu = acts.tile([128, 2, UNIT], BF16, tag="uu")
            mm_unit_dr("tab", lambda sl: ff2[:, :, sl], AF.Tanh, 0.5, uu,
                       cols)
            # out = hx + t*(ic+rc-hx), t = 0.5*u + 0.5
            ti = tmp.tile([128, 2, UNIT], BF16, tag="ti")
            nc.vector.tensor_scalar(ti[:, :, 0:cols], uu[:, :, 0:cols],
                                    0.5, 0.5, ALU.mult, ALU.add)
            pT = tmp.tile([128, 2, UNIT], BF16, tag="pT")
            nc.vector.tensor_mul(pT[:, :, 0:cols], ti[:, :, 0:cols], dT)
            nc.vector.tensor_add(hT_u, pT[:, :, 0:cols], hxu)
            for dst, src in stores:
                nc.sync.dma_start(dst, src)

        pending = None
        for s in range(n_slab):
            csl = slice(s * slab, (s + 1) * slab)
            xT = io_x.tile([128, slab], BF16, tag="xT")
            # unit-major layout: a unit's [128, 2, UNIT] slice is contiguous,
            # so the DVE combine ops stay in their 2x/4x perf modes
            hxT = io_hx.tile([128, n_unit, 2, UNIT], BF16, tag="hxT")
            hxT8 = io_hx.tile([128, 2, slab], FP8, tag="hxT8")
            if s == 0:
                # trigger order is latency-critical: weights, then unit 0's
                # matmul inputs, then everything else; bf16 hx (combine-only)
                # loads last
                nc.sync.dma_start(w_sb[:], w_d[:])
                nc.sync.dma_start(w8_sb[:], w8_d[:])
                chunks = [(0, 512), (512, 512)] + [
                    (c, UNIT) for c in range(UNIT, slab, UNIT)]
                for q0, qc in chunks:
                    qsl = slice(q0, q0 + qc)
                    nc.sync.dma_start(xT[:, qsl], xT_d[:, qsl])
                    nc.sync.dma_start(hxT8[:, 0, qsl], hxT8_d[0, :, qsl])
                    nc.sync.dma_start(hxT8[:, 1, qsl], hxT8_d[1, :, qsl])
                for q in range(n_unit):
                    gsl = slice(q * UNIT, (q + 1) * UNIT)
                    nc.sync.dma_start(hxT[:, q, 0, :], hxT_d[0, :, gsl])
                    nc.sync.dma_start(hxT[:, q, 1, :], hxT_d[1, :, gsl])
                nc.sync.dma_start(b_sb[:], b_d[:])
            else:
                nc.sync.dma_start(xT[:], xT_d[:, csl])
                nc.sync.dma_start(hxT8[:, 0, :], hxT8_d[0, :, csl])
                nc.sync.dma_start(hxT8[:, 1, :], hxT8_d[1, :, csl])
                nc.sync.dma_start(
                    hxT[:, :, 0, :],
                    hxT_d[0, :, csl].rearrange("p (u c) -> p u c", u=n_unit))
                nc.sync.dma_start(
                    hxT[:, :, 1, :],
                    hxT_d[1, :, csl].rearrange("p (u c) -> p u c", u=n_unit))
            hT = io_out.tile([128, n_unit, 2, UNIT], BF16, tag="hT")

            # variable unit plan: small first units (earlier first ACTIVATE)
            # and small last units (shorter end-of-kernel tail chain)
            if s == 0 and slab >= 2 * UNIT:
                plan = [(0, 512), (512, 512)] + [
                    (c, UNIT) for c in range(UNIT, slab, UNIT)]
            elif s == n_slab - 1 and slab >= 2 * UNIT:
                plan = [(c, UNIT) for c in range(0, slab - UNIT, UNIT)] + [
                    (slab - UNIT, 512), (slab - 512, 512)]
            else:
                plan = [(c, UNIT) for c in range(0, slab, UNIT)]

            for c0, cols in plan:
                u = c0 // UNIT
                off = c0 % UNIT
                x_src = lambda sl, c0=c0: xT[:, c0 + sl.start:c0 + sl.stop]
                hx8_src = lambda sl, c0=c0: hxT8[:, :, c0 + sl.start:
                                                 c0 + sl.stop]

                ic = acts.tile([128, 2, UNIT], BF16, tag="ic")
                mm_unit("ic", [x_src], AF.Gelu, 1.0, ic, cols)
                ff1 = acts.tile([128, 2, UNIT], FP8, tag="ff1")
                mm_unit_dr("ff1", hx8_src, AF.Gelu, 1.0, ff1, cols,
                           extra=(0, x_src))
                rc = acts.tile([128, 2, UNIT], BF16, tag="rc")
                mm_unit_dr("rc", hx8_src, AF.Gelu, 1.0, rc, cols)
                ff2 = acts.tile([128, 2, UNIT], FP8, tag="ff2")
                mm_unit_dr("ff2", lambda sl: ff1[:, :, sl], AF.Gelu, 1.0,
                           ff2, cols)

                # s/d only need ic/rc/hx — issue them ahead of the deferred
                # tail so the end-of-kernel combine chain is short
                hxu = hxT[:, u, :, off:off + cols]
                hTu = hT[:, u, :, off:off + cols]
                sT = tmp.tile([128, 2, UNIT], BF16, tag="sT")
                nc.vector.tensor_add(sT[:, :, 0:cols], ic[:, :, 0:cols],
                                     rc[:, :, 0:cols])
                dT = tmp.tile([128, 2, UNIT], BF16, tag="dT")
                nc.vector.tensor_sub(dT[:, :, 0:cols], sT[:, :, 0:cols], hxu)

                if pending is not None:
                    stage_tail(pending)
                gsl = slice(s * slab + c0, s * slab + c0 + cols)
                stores = [(out_d[0, :, gsl], hT[:, u, 0, off:off + cols]),
                          (out_d[1, :, gsl], hT[:, u, 1, off:off + cols])]
                pending = (ff2, dT[:, :, 0:cols], hxu, hTu, stores, cols)
        stage_tail(pending)
    nc.finalize()
    return nc


_NC_CACHE: dict = {}


def _get_nc(b_core: int, slab: int, zero_bias: bool) -> bass.Bass:
    key = (b_core, slab, zero_bias)
    if key not in _NC_CACHE:
        _NC_CACHE[key] = build_nc(b_core, slab, zero_bias)
    return _NC_CACHE[key]


def _prep_host(W_ff1, b_ff1, W_ff2, b_ff2, W_ta, b_ta, W_tb, b_tb,
               W_in, b_in, input_b, W_r, r_b):
    f32 = lambda a: np.asarray(a, dtype=np.float32)
    weights = {
        "ff1": f32(W_ff1),
        "ic": f32(W_in),
        "rc": f32(W_r),
        "ff2": f32(W_ff2),
        "tab": f32(W_ta) + f32(W_tb),
    }
    biases = {
        "ff1": f32(b_ff1),
        "ic": f32(b_in) + f32(input_b),
        "rc": f32(r_b),
        "ff2": f32(b_ff2),
        "tab": 0.5 * (f32(b_ta) + f32(b_tb)),
    }
    wstack = np.zeros([N_WCH, 128, 128], dtype=NP_BF16)
    for layer in LAYERS:
        W = weights[layer]
        for k in range(KCH[layer]):
            for m in range(2):
                ci = W_BASE[layer] + 2 * k + m
                wstack[ci] = np.ascontiguousarray(
                    W[m * 128:(m + 1) * 128, k * 128:(k + 1) * 128].T
                ).astype(NP_BF16)
    w8stack = np.zeros([N_W8, 128, 256], dtype=NP_FP8)
    for layer, kpair in DR_PAIRS:
        W = weights[layer]
        for m in range(2):
            pi = P8[(layer, m)]
            for t, k in enumerate(kpair):
                w8stack[pi][:, t * 128:(t + 1) * 128] = np.ascontiguousarray(
                    W[m * 128:(m + 1) * 128, k * 128:(k + 1) * 128].T
                ).astype(NP_FP8)
    bstack = np.zeros([128, 10], dtype=np.float32)
    for li, layer in enumerate(LAYERS):
        for m in range(2):
            bstack[:, 2 * li + m] = biases[layer][m * 128:(m + 1) * 128]
    zero_bias = not np.any(bstack)
    # pre-swizzle partition-major for contiguous DMA
    wstack = np.ascontiguousarray(wstack.transpose(1, 0, 2).reshape(128, -1))
    w8stack = np.ascontiguousarray(w8stack.transpose(1, 0, 2).reshape(128, -1))
    return wstack, w8stack, bstack, zero_bias


def _run(inputs: dict, b_core: int = B_CORE, r: int = R, n_cores: int = N_CORES,
         **run_kwargs):
    wstack, w8stack, bstack, zero_bias = _prep_host(
        inputs["W_ff1"], inputs["b_ff1"], inputs["W_ff2"], inputs["b_ff2"],
        inputs["W_ta"], inputs["b_ta"], inputs["W_tb"], inputs["b_tb"],
        inputs["W_in"], inputs["b_in"], inputs["input_b"], inputs["W_r"],
        inputs["r_b"])
    # host-side transpose to feature-major bf16 (+ fp8 copy of hx)
    hx_f32 = np.asarray(inputs["hx"], dtype=np.float32)
    xT = np.asarray(inputs["x"], dtype=NP_BF16).T          # [128, B]
    hxT = hx_f32.astype(NP_BF16).T                         # [256, B]
    n_rows = xT.shape[1]
    hxT = hxT.reshape(2, 128, n_rows)
    hxT8 = hx_f32.astype(NP_FP8).T.reshape(2, 128, n_rows)

    nc = _get_nc(b_core, r, zero_bias)
    in_maps = []
    for c in range(n_cores):
        sl = slice(c * b_core, (c + 1) * b_core)
        in_maps.append({
            "xT": np.ascontiguousarray(xT[:, sl]),
            "hxT": np.ascontiguousarray(hxT[:, :, sl]),
            "hxT8": np.ascontiguousarray(hxT8[:, :, sl]),
            "wstack": wstack,
            "w8stack": w8stack,
            "bstack": bstack,
        })
    res = run_bass_kernel_spmd(nc, in_maps, list(range(n_cores)), **run_kwargs)
    # de-transpose: out_d[m, p, b] is feature (m*128+p) of row b
    outs = [m["out"].transpose(2, 0, 1).reshape(b_core, H).astype(np.float32)
            for m in res.results]
    out = np.concatenate(outs, axis=0)
    return out, res


def kernel(**inputs):
    out, _ = _run(inputs)
    return (out, out)
